# revision 1
# baseline (speedup 1.0000x reference)
"""Grouped-Query Attention forward pass on 8 Trainium2 NeuronCores.

Sharding: 2-way data parallel over batch x 4-way tensor parallel over KV
head groups. Core c = 4*b + g handles batch b and KV group g (4 query
heads + 1 KV head). Each core computes a partial o-projection output
(its head group's contribution, full [S, D]); the host sums the 4
partials per batch (the "all-reduce" is a host-side gather+sum since we
must return a full numpy output anyway).

Device kernel per core (all matmuls in float32r - full PE rate, ~1.6e-4
rounding):
  Phase 1: q = x @ Wq_shard, [k|v] = x @ Wkv_shard (contraction over D
    via xT tiles), per-head RMS norm + RoPE on q and k (norm weights and
    the 1/sqrt(hd) attention scale are folded into host-precomputed
    cos/sin tables), PE-transpose q,k -> qT, kT. v stays natural.
  Phase 2 (per head, per 512-wide query chunk): scoresT = kT_slice.T @ qT
    -> exp (ACT, PSUM->SBUF, no max subtraction: RMS-normed q,k bound
    |score| <= sqrt(hd) so exp is safe in fp32) -> AV accumulation
    (lhsT = v natural) and ones-matmul denominator, then normalize at
    PSUM eviction with a DMA-broadcast reciprocal.
  Phase 3: o_partial = concat_heads(out) @ Wo_shard rows (lhsT = outT
    from phase 2, already transposed).
"""

import sys

sys.path.insert(0, "/opt/trn_rl_repo")

import numpy as np

import concourse.bass as bass
import concourse.tile as tile
from concourse import bacc, mybir
from concourse.bass_utils import run_bass_kernel_spmd
from concourse.masks import make_identity

F32 = mybir.dt.float32
F32R = mybir.dt.float32r
AF = mybir.ActivationFunctionType

B = 2
S = 2048
D = 2048
NH = 16
NKV = 4
HD = 128
G = NH // NKV  # 4 query heads per KV head / per core
DQ = G * HD  # 512 query dims per core
EPS = 1e-6
ROPE_BASE = 10000.0

NT = S // 128  # 16 sequence tiles
ND = D // 128  # 16 contraction slices
QC = 4  # query chunks of 512
KT = S // 128  # 16 key tiles

_cached_nc = None
last_results = None  # BassKernelResults of the most recent run (for test.py)


def _build_program():
    nc = bacc.Bacc("TRN2", target_bir_lowering=False, debug=False)

    xt = nc.dram_tensor("xt", [D, S], F32R, kind="ExternalInput").ap()
    wq = nc.dram_tensor("wq", [D, DQ], F32R, kind="ExternalInput").ap()
    wkv = nc.dram_tensor("wkv", [D, 2 * HD], F32R, kind="ExternalInput").ap()
    wo = nc.dram_tensor("wo", [DQ, D], F32R, kind="ExternalInput").ap()
    cq = nc.dram_tensor("cq", [S, HD], F32, kind="ExternalInput").ap()
    sq = nc.dram_tensor("sq", [S, HD], F32, kind="ExternalInput").ap()
    ck = nc.dram_tensor("ck", [S, HD], F32, kind="ExternalInput").ap()
    sk = nc.dram_tensor("sk", [S, HD], F32, kind="ExternalInput").ap()
    ones = nc.dram_tensor("ones", [128, 1], F32R, kind="ExternalInput").ap()
    o = nc.dram_tensor("o", [S, D], F32, kind="ExternalOutput").ap()

    def dram3(t, pstep, bstep, nb, line):
        # [128 partitions, nb, line] view of a DRAM matrix
        return lambda off: bass.AP(
            tensor=t.tensor, offset=off, ap=[[pstep, 128], [bstep, nb], [1, line]]
        )

    with tile.TileContext(nc) as tc:
        from contextlib import ExitStack

        with ExitStack() as ctx:
            persist = ctx.enter_context(tc.tile_pool(name="persist", bufs=1))

            # persistent SBUF tensors
            qt = [persist.tile([128, S], F32R, name=f"qt{h}", tag=f"qt{h}") for h in range(G)]
            kt_sb = persist.tile([128, S], F32R, tag="kt")
            v_sb = persist.tile([128, KT, HD], F32R, tag="v")
            outt = [persist.tile([128, S], F32R, name=f"outt{h}", tag=f"outt{h}") for h in range(G)]
            ident = persist.tile([128, 128], F32, tag="ident")
            make_identity(nc, ident[:])
            ones_sb = persist.tile([128, 1], F32R, tag="ones")
            nc.sync.dma_start(ones_sb[:], ones)
            eps_sb = persist.tile([128, 1], F32, tag="eps")
            nc.vector.memset(eps_sb[:], EPS)

            # ---------------- Phase 1: projections + RMS + RoPE ----------------
            with ExitStack() as p1:
                p1.enter_context(nc.named_scope("p1_proj"))
                wpool = p1.enter_context(tc.tile_pool(name="w1", bufs=1))
                xpool = p1.enter_context(tc.tile_pool(name="xcol", bufs=3))
                rope = p1.enter_context(tc.tile_pool(name="rope", bufs=3))
                small = p1.enter_context(tc.tile_pool(name="small", bufs=4))
                ps1 = p1.enter_context(tc.tile_pool(name="ps1", bufs=3, space="PSUM"))
                pst = p1.enter_context(tc.tile_pool(name="pst", bufs=2, space="PSUM"))

                wq_sb = wpool.tile([128, ND, DQ], F32R, tag="wq")
                wkv_sb = wpool.tile([128, ND, 2 * HD], F32R, tag="wkv")
                # wq[d, n]: partition = d % 128, blocks = d // 128
                for i in range(4):
                    nc.sync.dma_start(
                        wq_sb[:, 4 * i : 4 * i + 4, :],
                        dram3(wq, DQ, 128 * DQ, 4, DQ)(4 * i * 128 * DQ),
                    )
                nc.sync.dma_start(wkv_sb[:], dram3(wkv, 2 * HD, 128 * 2 * HD, ND, 2 * HD)(0))

                cq_sb = wpool.tile([128, NT, HD], F32, tag="cq")
                sq_sb = wpool.tile([128, NT, HD], F32, tag="sq")
                ck_sb = wpool.tile([128, NT, HD], F32, tag="ck")
                sk_sb = wpool.tile([128, NT, HD], F32, tag="sk")
                for t, t_sb in ((cq, cq_sb), (sq, sq_sb), (ck, ck_sb), (sk, sk_sb)):
                    nc.sync.dma_start(t_sb[:], dram3(t, HD, 128 * HD, NT, HD)(0))

                for st in range(NT):
                    xcol = xpool.tile([128, ND, 128], F32R, tag="xcol")
                    # xt[d, s] slice s in [st*128, +128): partition d%128
                    for i in range(4):
                        nc.sync.dma_start(
                            xcol[:, 4 * i : 4 * i + 4, :],
                            dram3(xt, S, 128 * S, 4, 128)(4 * i * 128 * S + st * 128),
                        )

                    q_ps = ps1.tile([128, DQ], F32, tag="q_ps")
                    kv_ps = ps1.tile([128, 2 * HD], F32, tag="kv_ps")
                    for ds in range(ND):
                        nc.tensor.matmul(
                            q_ps[:],
                            xcol[:, ds, :],
                            wq_sb[:, ds, :],
                            start=(ds == 0),
                            stop=(ds == ND - 1),
                        )
                    for ds in range(ND):
                        nc.tensor.matmul(
                            kv_ps[:],
                            xcol[:, ds, :],
                            wkv_sb[:, ds, :],
                            start=(ds == 0),
                            stop=(ds == ND - 1),
                        )

                    # v: straight copy to natural layout
                    nc.scalar.copy(v_sb[:, st, :], kv_ps[:, HD : 2 * HD])

                    # RMS norm + RoPE per head chunk (4 q heads + 1 k)
                    for hc in range(G + 1):
                        if hc < G:
                            src = q_ps[:, hc * HD : (hc + 1) * HD]
                            cos_t, sin_t = cq_sb[:, st, :], sq_sb[:, st, :]
                        else:
                            src = kv_ps[:, 0:HD]
                            cos_t, sin_t = ck_sb[:, st, :], sk_sb[:, st, :]

                        sqv = small.tile([128, HD], F32, tag="sqv")
                        ssq = small.tile([128, 1], F32, tag="ssq")
                        nc.scalar.activation(
                            sqv[:], src, AF.Square, accum_out=ssq[:]
                        )
                        rms = small.tile([128, 1], F32, tag="rms")
                        nc.scalar.activation(
                            rms[:], ssq[:], AF.Sqrt, bias=eps_sb[:], scale=1.0 / HD
                        )
                        nc.vector.reciprocal(rms[:], rms[:])

                        qh = rope.tile([128, HD], F32, tag="qh")
                        nc.vector.tensor_scalar_mul(qh[:], src, rms[:])

                        # rotate-half view: qh[p, (f+64) % 128]
                        rot = bass.AP(
                            tensor=qh[:].tensor,
                            offset=qh[:].offset + 64,
                            ap=[qh[:].ap[0], [-64, 2], [1, 64]],
                        )
                        t1 = rope.tile([128, HD], F32, tag="t1")
                        t2 = rope.tile([128, HD], F32, tag="t2")
                        nc.vector.tensor_mul(t1[:], qh[:], cos_t)
                        nc.vector.tensor_mul(
                            t2[:].rearrange("p (a b) -> p a b", a=2),
                            rot,
                            sin_t.rearrange("p (a b) -> p a b", a=2),
                        )
                        qr = rope.tile([128, HD], F32, tag="qr")
                        nc.vector.tensor_add(qr[:], t1[:], t2[:])

                        # transpose -> qT / kT
                        tr_ps = pst.tile([128, 128], F32, tag="tr")
                        nc.tensor.transpose(tr_ps[:], qr[:], ident[:])
                        dst = qt[hc] if hc < G else kt_sb
                        nc.scalar.copy(dst[:, st * 128 : (st + 1) * 128], tr_ps[:])

            # wo prefetch: phase-1 pools are released; load now so phase 3
            # never waits on this 4MB DMA.
            wo_pool = ctx.enter_context(tc.tile_pool(name="wo_pool", bufs=1))
            wo_sb = wo_pool.tile([128, G, D], F32R, tag="wo")
            for i in range(2):
                nc.sync.dma_start(
                    wo_sb[:, 2 * i : 2 * i + 2, :],
                    dram3(wo, D, 128 * D, 2, D)(2 * i * 128 * D),
                )

            # ---------------- Phase 2: attention ----------------
            with ExitStack() as p2:
                p2.enter_context(nc.named_scope("p2_attn"))
                epool = p2.enter_context(tc.tile_pool(name="exp", bufs=2))
                dpool = p2.enter_context(tc.tile_pool(name="den", bufs=3))
                drpool = p2.enter_context(tc.tile_pool(name="dend", bufs=3, space="DRAM"))
                ps_s = p2.enter_context(tc.tile_pool(name="ps_s", bufs=4, space="PSUM"))
                ps_av = p2.enter_context(tc.tile_pool(name="ps_av", bufs=2, space="PSUM"))
                ps_dn = p2.enter_context(tc.tile_pool(name="ps_dn", bufs=2, space="PSUM"))

                for h in range(G):
                    for qc in range(QC):
                        qsl = slice(qc * 512, (qc + 1) * 512)
                        exp_sb = epool.tile([128, KT, 512], F32R, tag="exp")
                        av_ps = ps_av.tile([128, 512], F32, tag="av")
                        den_ps = ps_dn.tile([1, 512], F32, tag="den")
                        for kt in range(KT):
                            s_ps = ps_s.tile([128, 512], F32, tag="s")
                            nc.tensor.matmul(
                                s_ps[:],
                                kt_sb[:, kt * 128 : (kt + 1) * 128],
                                qt[h][:, qsl],
                                start=True,
                                stop=True,
                            )
                            nc.scalar.activation(exp_sb[:, kt, :], s_ps[:], AF.Exp)
                            nc.tensor.matmul(
                                den_ps[:],
                                ones_sb[:],
                                exp_sb[:, kt, :],
                                start=(kt == 0),
                                stop=(kt == KT - 1),
                            )
                            nc.tensor.matmul(
                                av_ps[:],
                                v_sb[:, kt, :],
                                exp_sb[:, kt, :],
                                start=(kt == 0),
                                stop=(kt == KT - 1),
                            )

                        den_sb = dpool.tile([1, 512], F32, tag="den_sb")
                        nc.scalar.copy(den_sb[:], den_ps[:])
                        nc.vector.reciprocal(den_sb[:], den_sb[:])
                        den_dr = drpool.tile([1, 512], F32, tag="den_dr")
                        nc.sync.dma_start(den_dr[:], den_sb[:])
                        rbc = dpool.tile([128, 512], F32, tag="rbc")
                        nc.sync.dma_start(
                            rbc[:],
                            bass.AP(
                                tensor=den_dr[:].tensor,
                                offset=den_dr[:].offset,
                                ap=[[0, 128], [1, 512]],
                            ),
                        )
                        nc.vector.tensor_mul(outt[h][:, qsl], av_ps[:], rbc[:])

            # ---------------- Phase 3: output projection ----------------
            with ExitStack() as p3:
                p3.enter_context(nc.named_scope("p3_oproj"))
                opool = p3.enter_context(tc.tile_pool(name="osb", bufs=3))
                ps_o = p3.enter_context(tc.tile_pool(name="ps_o", bufs=4, space="PSUM"))

                for st in range(NT):
                    o_sb = opool.tile([128, 4, 512], F32, tag="o_sb")
                    for dc in range(4):
                        op_ps = ps_o.tile([128, 512], F32, tag="op")
                        for h in range(G):
                            nc.tensor.matmul(
                                op_ps[:],
                                outt[h][:, st * 128 : (st + 1) * 128],
                                wo_sb[:, h, dc * 512 : (dc + 1) * 512],
                                start=(h == 0),
                                stop=(h == G - 1),
                            )
                        nc.scalar.copy(o_sb[:, dc, :], op_ps[:])
                    nc.sync.dma_start(
                        bass.AP(
                            tensor=o.tensor,
                            offset=st * 128 * D,
                            ap=[[D, 128], [1, D]],
                        ),
                        o_sb[:].rearrange("p a b -> p (a b)"),
                    )

    nc.compile()
    return nc


def _rope_tables(qw, kw):
    """Folded cos/sin tables. RoPE rotation with rotate-half; per-head RMS
    norm weight w and the attention scale sc are folded in:
      out[d] = qhat[d]*w[d]*cos[d]*sc + qhat[(d+64)%128]*(sgn)*w[(d+64)%128]*sin[d]*sc
    where sgn = -1 for d < 64 (rotate-half negates the upper half moved down).
    """
    inv_freq = 1.0 / (ROPE_BASE ** (np.arange(0, HD, 2, dtype=np.float32) / HD))
    t = np.arange(S, dtype=np.float32)
    freqs = np.outer(t, inv_freq).astype(np.float32)  # [S, 64]
    emb = np.concatenate([freqs, freqs], axis=1)  # [S, 128]
    cos = np.cos(emb).astype(np.float32)
    sin = np.sin(emb).astype(np.float32)

    sgn = np.where(np.arange(HD) < 64, np.float32(-1.0), np.float32(1.0))
    wshift_q = np.roll(qw, -64)  # w[(d+64)%128]
    wshift_k = np.roll(kw, -64)
    sc = np.float32(1.0 / np.sqrt(HD))
    cq = cos * qw[None, :] * sc
    sq_ = sin * (sgn * wshift_q)[None, :] * sc
    ck = cos * kw[None, :]
    sk_ = sin * (sgn * wshift_k)[None, :]
    return (
        np.ascontiguousarray(cq, dtype=np.float32),
        np.ascontiguousarray(sq_, dtype=np.float32),
        np.ascontiguousarray(ck, dtype=np.float32),
        np.ascontiguousarray(sk_, dtype=np.float32),
    )


def kernel(x, Wq, Wk, Wv, Wo, q_norm_w, k_norm_w):
    global _cached_nc, last_results
    x = np.asarray(x, dtype=np.float32)
    Wq = np.asarray(Wq, dtype=np.float32)
    Wk = np.asarray(Wk, dtype=np.float32)
    Wv = np.asarray(Wv, dtype=np.float32)
    Wo = np.asarray(Wo, dtype=np.float32)
    qw = np.asarray(q_norm_w, dtype=np.float32)
    kw = np.asarray(k_norm_w, dtype=np.float32)

    if _cached_nc is None:
        _cached_nc = _build_program()
    nc = _cached_nc

    cq, sq_, ck, sk_ = _rope_tables(qw, kw)
    ones = np.ones((128, 1), dtype=np.float32)

    in_maps = []
    for c in range(8):
        b, g = divmod(c, 4)
        in_maps.append(
            {
                "xt": np.ascontiguousarray(x[b].T),
                "wq": np.ascontiguousarray(Wq[:, g * DQ : (g + 1) * DQ]),
                "wkv": np.ascontiguousarray(
                    np.concatenate(
                        [
                            Wk[:, g * HD : (g + 1) * HD],
                            Wv[:, g * HD : (g + 1) * HD],
                        ],
                        axis=1,
                    )
                ),
                "wo": np.ascontiguousarray(Wo[g * DQ : (g + 1) * DQ, :]),
                "cq": cq,
                "sq": sq_,
                "ck": ck,
                "sk": sk_,
                "ones": ones,
            }
        )

    last_results = run_bass_kernel_spmd(nc, in_maps, core_ids=list(range(8)))

    out = np.zeros((B, S, D), dtype=np.float32)
    for c in range(8):
        b = c // 4
        out[b] += last_results.results[c]["o"]
    return out

